# revision 1
# baseline (speedup 1.0000x reference)
"""GAT layer kernel for Trainium2, 8 NeuronCores — v3 (dma_gather).

Design:
  - Table whaug[r] (DRAM, stride 384 f16): r = per-core-ROTATED (tile,slot)
    row id; row = [Wh_int(256) (col 2d+h), t0, t1, s0, s1, pad...].
    Per-core rotation puts each core's own nodes at rows 0..6271, so the
    self-row (fb) loads are direct DMAs and all dma_gather indices fit the
    layout below.
  - dma_gather (gpsimd ucode) fetches dst rows: int16 indices < 32768, so
    rows are split A (< 32768) / B (>= 32768, gathered from an offset
    table view); per tile the slot layout is [CA A-chunks | CB B-chunks]
    (row-0 padded); <=1024 indices per instruction (HW carveout).
  - s[src] per edge: one-hot built on DVE (2x mode), each chunk
    PE-transposed (copies alternate DVE/ACT), then C' tiny matmuls
    rhs=own s-pairs.
  - agg: eps=1e-20 f32 self-edge matmul + C' f16 matmuls [oneh chunk]
    @ [p*Wh rows]; den rides in cols 256:258.
"""

import math
import sys
from dataclasses import dataclass

import numpy as np

sys.path.insert(0, "/opt/trn_rl_repo")

import concourse.bass as bass
import concourse.mybir as mybir
import concourse.tile as tile
from concourse import bacc
from concourse.masks import make_identity
from concourse.bass_utils import run_bass_kernel_spmd

N_NODES = 50000
IN_DIM = 256
OUT_DIM = 128
P = 128
TW = 384  # table row stride (f16 elems); payload in cols 0:260
SPLIT = 32768
SHIFT = 4.0
EPS_SELF = 1e-20

F32 = mybir.dt.float32
F16 = mybir.dt.float16
I16 = mybir.dt.int16


@dataclass(frozen=True)
class Cfg:
    n_nodes: int
    n_cores: int
    CA: int
    CB: int
    span_tiles: int = 8
    ogrp: int = 8
    reps: int = 1

    @property
    def C(self):
        return self.CA + self.CB

    @property
    def nodes_per_core(self):
        return self.n_nodes // self.n_cores

    @property
    def ntiles(self):
        return (self.nodes_per_core + P - 1) // P

    @property
    def npad(self):
        return self.n_cores * self.ntiles * P


def _ap_expand(ap, dims):
    return bass.AP(ap.tensor, ap.offset, [list(ap.ap[0])] + [[s, c] for s, c in dims])


def _wrap_rep(idxs):
    """flat int16 list (len%128==0) -> [128, n/16] wrapped + replicated."""
    n = len(idxs)
    blk = np.asarray(idxs, dtype=np.int16).reshape(n // 16, 16).T
    return np.tile(blk, (8, 1))


def host_prep(x, edge_index, W_w, W_b, a, n_cores=8):
    x = np.asarray(x, dtype=np.float32)
    edge_index = np.asarray(edge_index)
    W_w = np.asarray(W_w, dtype=np.float32)
    W_b = np.asarray(W_b, dtype=np.float32)
    a = np.asarray(a, dtype=np.float32)
    assert np.abs(W_b).max() == 0.0

    n_nodes, in_dim = x.shape
    D = OUT_DIM
    n_edges = edge_index.shape[1]

    a_src, a_dst = a[:D], a[D:]
    W_int = (
        W_w.reshape(in_dim, 2, D).transpose(0, 2, 1).reshape(in_dim, 2 * D)
    )
    ws0 = W_w[:, 0:D] @ a_src
    ws1 = W_w[:, D:] @ a_src
    wt0 = W_w[:, 0:D] @ a_dst
    wt1 = W_w[:, D:] @ a_dst
    wbig = np.concatenate(
        [W_int, wt0[:, None], wt1[:, None], ws0[:, None], ws1[:, None]], axis=1
    ).astype(np.float16)  # [in_dim, 260]

    src = np.asarray(edge_index[0], dtype=np.int64)
    dst = np.asarray(edge_index[1], dtype=np.int64)

    npc = n_nodes // n_cores
    ntiles = (npc + P - 1) // P

    # LPT: node -> (global tile, slot), balancing per-tile edge count
    import heapq

    ntile_tot = n_cores * ntiles
    deg_all = np.bincount(src, minlength=n_nodes)
    order_n = np.argsort(-deg_all, kind="stable")
    heap = [(0, t) for t in range(ntile_tot)]
    heapq.heapify(heap)
    fill = np.zeros(ntile_tot, dtype=np.int64)
    node_tile = np.zeros(n_nodes, dtype=np.int64)
    node_slot = np.zeros(n_nodes, dtype=np.int64)
    for n in order_n:
        while True:
            w, t = heapq.heappop(heap)
            if fill[t] < P:
                break
        node_tile[n] = t
        node_slot[n] = fill[t]
        fill[t] += 1
        if fill[t] < P:
            heapq.heappush(heap, (w + int(deg_all[n]), t))

    grow = node_tile * P + node_slot  # node -> global table row
    npad = ntile_tot * P

    # per-edge: owner core = src's core; global row ids
    ecore = node_tile[src] // ntiles
    etile_l = node_tile[src] % ntiles  # local tile on owner core
    eslot = node_slot[src]
    edst_grow = grow[dst]

    # per-core/per-tile/per-class edge counts -> CA, CB (global maxima)
    erow_l = np.zeros(n_edges, dtype=np.int64)
    for k in range(n_cores):
        m = ecore == k
        erow_l[m] = (edst_grow[m] - k * ntiles * P) % npad
    eclassB = erow_l >= SPLIT

    cntA = np.zeros((n_cores, ntiles), dtype=np.int64)
    cntB = np.zeros((n_cores, ntiles), dtype=np.int64)
    np.add.at(cntA, (ecore[~eclassB], etile_l[~eclassB]), 1)
    np.add.at(cntB, (ecore[eclassB], etile_l[eclassB]), 1)
    CA = int(math.ceil(cntA.max() / P))
    CB = int(math.ceil(cntB.max() / P))
    cfg = Cfg(n_nodes=n_nodes, n_cores=n_cores, CA=CA, CB=CB)
    C = cfg.C

    # iota constants
    iota_sc = np.broadcast_to(
        np.arange(P, dtype=np.float16)[None, :, None], (P, P, C)
    ).copy()  # [p, s, c] = s  (for oneh)
    shared = {"wbig": wbig, "iota_sc": iota_sc}
    per_core = []
    for k in range(n_cores):
        # rotated node -> local row
        lrow_node = (grow - k * ntiles * P) % npad  # node -> local row
        # xT rotated: local row r holds node with lrow_node == r
        xT = np.zeros((in_dim, npad), dtype=np.float16)
        own = lrow_node  # [n_nodes]
        xT[:, own] = x.T.astype(np.float16)

        m = ecore == k
        et, es = etile_l[m], eslot[m]
        er = erow_l[m]
        eB = eclassB[m]

        # slot assignment within tile: A edges then B edges
        srcL = np.full((ntiles, C * P), -1.0, dtype=np.float16)
        idxA = np.zeros((ntiles, CA * P), dtype=np.int64)
        idxB = np.zeros((ntiles, CB * P), dtype=np.int64)
        order = np.lexsort((er, eB, et))  # group by tile, class A first
        et, es, er, eB = et[order], es[order], er[order], eB[order]
        for t in range(ntiles):
            tm = et == t
            rA = er[tm & ~eB]
            sA = es[tm & ~eB]
            rB = er[tm & eB] - SPLIT
            sB = es[tm & eB]
            idxA[t, : len(rA)] = rA
            idxB[t, : len(rB)] = rB
            # slot s of region -> (chunk s//P within region, partition s%P)
            a_sl = np.arange(len(rA))
            srcL[t, (a_sl // P) * P + a_sl % P] = sA
            b_sl = np.arange(len(rB))
            srcL[t, CA * P + (b_sl // P) * P + b_sl % P] = sB

        # dma_gather order: idx i -> out (partition i%128, block i//128);
        # slot (chunk c, partition p) = flat c*128+p = i  => identity order
        idxA16 = np.stack([_wrap_rep(idxA[t]) for t in range(ntiles)], axis=1)
        idxB16 = np.stack([_wrap_rep(idxB[t]) for t in range(ntiles)], axis=1)
        # [128, ntiles, n/16] -> [128, ntiles * n/16]
        idxA16 = np.ascontiguousarray(idxA16).reshape(P, -1)
        idxB16 = np.ascontiguousarray(idxB16).reshape(P, -1)

        # srcL per-slot in [P, ntiles*C] layout (partition = slot%P)
        srcL_pc = np.ascontiguousarray(
            srcL.reshape(ntiles, C, P).transpose(2, 0, 1)
        ).reshape(P, ntiles * C)
        mine = np.nonzero(node_tile // ntiles == k)[0]
        rows_k = lrow_node[mine]
        per_core.append(
            {
                "xT": xT,
                "idxA": idxA16.astype(np.int16),
                "idxB": idxB16.astype(np.int16),
                "srcL": srcL_pc,
                "_nodes": mine,
                "_rows": rows_k,
            }
        )
    return cfg, shared, per_core


def build_program(cfg: Cfg):
    CA, CB, C = cfg.CA, cfg.CB, cfg.C
    ntiles, npad = cfg.ntiles, cfg.npad
    OG = cfg.ogrp
    nc = bacc.Bacc("TRN2", target_bir_lowering=False, debug=False)

    xT_d = nc.dram_tensor("xT", [IN_DIM, npad], F16, kind="ExternalInput")
    wbig_d = nc.dram_tensor("wbig", [IN_DIM, 260], F16, kind="ExternalInput")
    iosc_d = nc.dram_tensor("iota_sc", [P, P, C], F16, kind="ExternalInput")
    idxA_d = nc.dram_tensor("idxA", [P, ntiles * CA * 8], I16, kind="ExternalInput")
    idxB_d = nc.dram_tensor("idxB", [P, ntiles * CB * 8], I16, kind="ExternalInput")
    srcL_d = nc.dram_tensor("srcL", [P, ntiles * C], F16, kind="ExternalInput")
    out_d = nc.dram_tensor("out", [ntiles * P, 2 * OUT_DIM], F16, kind="ExternalOutput")

    whaug_d = nc.dram_tensor("whaug", [npad, TW], F16)

    n_alltiles = npad // P

    with tile.TileContext(nc) as tc:
        with (
            tc.tile_pool(name="const", bufs=1) as constp,
            tc.tile_pool(name="xk", bufs=2) as xkp,
            tc.tile_pool(name="bld_ps", bufs=2, space="PSUM") as bldps,
            tc.tile_pool(name="augg", bufs=2) as auggp,
            tc.tile_pool(name="tr_ps", bufs=2, space="PSUM") as trps,
            tc.tile_pool(name="oneT", bufs=2) as oneTp,
            tc.tile_pool(name="fbg", bufs=2) as fbgp,
            tc.tile_pool(name="fb32", bufs=2) as fb32p,
            tc.tile_pool(name="gall", bufs=2) as gallp,
            tc.tile_pool(name="oneh", bufs=2) as onehp,
            tc.tile_pool(name="rhs", bufs=2) as rhsp,
            tc.tile_pool(name="p16", bufs=2) as p16p,
            tc.tile_pool(name="ework", bufs=2) as ep,
            tc.tile_pool(name="agg_ps", bufs=2, space="PSUM") as aggps,
            tc.tile_pool(name="s_ps", bufs=2, space="PSUM") as spsp,
            tc.tile_pool(name="og", bufs=2) as ogp,
        ):
            # ---------------- constants ----------------
            wb = constp.tile([P, 2, 260], F16, tag="wb")
            nc.sync.dma_start(
                out=wb[:], in_=wbig_d[:, :].rearrange("(kt kp) c -> kp kt c", kp=P)
            )
            iosc_t = constp.tile([P, P, C], F16, tag="iosc")
            nc.sync.dma_start(out=iosc_t[:], in_=iosc_d[:, :, :])
            idxA_t = constp.tile([P, ntiles * CA * 8], I16, tag="idxA")
            nc.sync.dma_start(out=idxA_t[:], in_=idxA_d[:, :])
            idxB_t = constp.tile([P, ntiles * CB * 8], I16, tag="idxB")
            nc.sync.dma_start(out=idxB_t[:], in_=idxB_d[:, :])
            srcL_t = constp.tile([P, ntiles * C], F16, tag="srcL")
            nc.sync.dma_start(out=srcL_t[:], in_=srcL_d[:, :])
            shift_t = constp.tile([P, 1], F32, tag="shift")
            nc.vector.memset(shift_t[:], -SHIFT)
            ident_t = constp.tile([P, P], F32, tag="ident")
            make_identity(nc, ident_t[:])
            ident16_t = constp.tile([P, P], F16, tag="ident16")
            nc.vector.tensor_copy(out=ident16_t[:], in_=ident_t[:])
            epsI_t = constp.tile([P, P], F32, tag="epsI")
            nc.vector.tensor_scalar(
                out=epsI_t[:], in0=ident_t[:], scalar1=EPS_SELF, scalar2=None,
                op0=mybir.AluOpType.mult,
            )

            # ---------------- phase 1: build whaug table ----------------
            GRP = 8
            n0 = 0
            ci = 0
            while n0 < n_alltiles:
                span = min(cfg.span_tiles, n_alltiles - n0)
                xk = xkp.tile([P, 2, cfg.span_tiles * P], F16, tag="xk")
                for kt in range(2):
                    nc.sync.dma_start(
                        out=xk[:, kt, 0 : span * P],
                        in_=xT_d[kt * P : (kt + 1) * P, n0 * P : (n0 + span) * P],
                    )
                g0 = 0
                while g0 < span:
                    grp = min(GRP, span - g0)
                    aug = auggp.tile([P, GRP, 260], F16, tag="aug")
                    for g in range(grp):
                        nt = g0 + g
                        ps = bldps.tile([P, 260], F32, tag="bld")
                        for kt in range(2):
                            nc.tensor.matmul(
                                out=ps[:],
                                lhsT=xk[:, kt, nt * P : (nt + 1) * P],
                                rhs=wb[:, kt, :],
                                start=(kt == 0),
                                stop=(kt == 1),
                            )
                        if ci % 2 == 0:
                            nc.vector.tensor_copy(out=aug[:, g, :], in_=ps[:])
                        else:
                            nc.scalar.activation(
                                out=aug[:, g, :], in_=ps[:],
                                func=mybir.ActivationFunctionType.Copy,
                            )
                        ci += 1
                    r0 = (n0 + g0) * P
                    nc.sync.dma_start(
                        out=whaug_d[r0 : r0 + grp * P, 0:260].rearrange(
                            "(g p) c -> p g c", p=P
                        ),
                        in_=aug[:, 0:grp, :],
                    )
                    g0 += grp
                n0 += span

            # ---------------- phase 2 ----------------
            og = fbg = None
            for t in [tt for _ in range(cfg.reps) for tt in range(ntiles)]:
                if t % OG == 0:
                    ow = min(OG, ntiles - t)
                    og = ogp.tile([P, OG, 2 * OUT_DIM], F16, tag="og")
                    fbg = fbgp.tile([P, OG, 260], F16, tag="fbg")
                    nc.sync.dma_start(
                        out=fbg[:, 0:ow, :],
                        in_=whaug_d[t * P : (t + ow) * P, 0:260].rearrange(
                            "(g p) c -> p g c", p=P
                        ),
                    )
                oi = t % OG

                # ---- gathers: A (2 instrs if CA>8) + B ----
                gall = gallp.tile([P, C, TW], F16, tag="gall")
                a_done = 0
                while a_done < CA:
                    an = min(8, CA - a_done)  # 8 chunks = 1024 idxs max
                    nc.gpsimd.dma_gather(
                        out_ap=gall[:, a_done : a_done + an, :],
                        in_ap=whaug_d[0:SPLIT, 0:TW],
                        idxs_ap=idxA_t[
                            :, (t * CA + a_done) * 8 : (t * CA + a_done + an) * 8
                        ],
                        num_idxs=an * P,
                        num_idxs_reg=an * P,
                        elem_size=TW,
                        elem_step=TW,
                    )
                    a_done += an
                b_done = 0
                while b_done < CB:
                    bn = min(8, CB - b_done)
                    nc.gpsimd.dma_gather(
                        out_ap=gall[:, CA + b_done : CA + b_done + bn, :],
                        in_ap=whaug_d[SPLIT:npad, 0:TW],
                        idxs_ap=idxB_t[
                            :, (t * CB + b_done) * 8 : (t * CB + b_done + bn) * 8
                        ],
                        num_idxs=bn * P,
                        num_idxs_reg=bn * P,
                        elem_size=TW,
                        elem_step=TW,
                    )
                    b_done += bn

                # ---- one-hot + s expansion (PE-transposed chunks) ----
                oneh = onehp.tile([P, P, C], F16, tag="oneh")
                nc.vector.tensor_tensor(
                    out=oneh[:],
                    in0=_ap_expand(srcL_t[:, t * C : (t + 1) * C], [(0, P), (1, C)]),
                    in1=iosc_t[:],
                    op=mybir.AluOpType.is_equal,
                )
                s_ps = spsp.tile([P, C, 2], F32, tag="s_ps")
                g0 = 0
                gi = 0
                while g0 < C:
                    gn = min(8, C - g0)
                    trp = trps.tile([P, 8, P], F16, tag="oneT_ps")
                    for c in range(g0, g0 + gn):
                        nc.tensor.transpose(
                            out=trp[:, c - g0, :], in_=oneh[:, :, c],
                            identity=ident16_t[:],
                        )
                    oneT = oneTp.tile([P, 8, P], F16, tag="oneT")
                    if gi % 2 == 0:
                        nc.vector.tensor_copy(
                            out=oneT[:, 0:gn, :], in_=trp[:, 0:gn, :]
                        )
                    else:
                        nc.scalar.activation(
                            out=oneT[:, 0:gn, :], in_=trp[:, 0:gn, :],
                            func=mybir.ActivationFunctionType.Copy,
                        )
                    for c in range(g0, g0 + gn):
                        nc.tensor.matmul(
                            out=s_ps[:, c, :],
                            lhsT=oneT[:, c - g0, :],
                            rhs=fbg[:, oi, 258:260],
                            start=True,
                            stop=True,
                        )
                    g0 += gn
                    gi += 1

                # ---- scores ----
                e_t = ep.tile([P, C, 2], F32, tag="e_t")
                nc.vector.tensor_tensor(
                    out=e_t[:],
                    in0=s_ps[:],
                    in1=gall[:, :, 256:258],
                    op=mybir.AluOpType.add,
                )
                e_s = ep.tile([P, C, 2], F32, tag="e_s")
                nc.vector.tensor_scalar(
                    out=e_s[:], in0=e_t[:], scalar1=0.2, scalar2=None,
                    op0=mybir.AluOpType.mult,
                )
                lr_t = ep.tile([P, C, 2], F32, tag="lr_t")
                nc.vector.tensor_tensor(
                    out=lr_t[:], in0=e_t[:], in1=e_s[:], op=mybir.AluOpType.max,
                )
                p16 = p16p.tile([P, C, 2], F16, tag="p16")
                nc.scalar.activation(
                    out=p16[:], in_=lr_t[:],
                    func=mybir.ActivationFunctionType.Exp,
                    bias=shift_t[:, 0:1],
                )

                # ---- rhs = p * gall(Wh cols) ; den cols = p ----
                rhs = rhsp.tile([P, C, 258], F16, tag="rhs")
                nc.vector.tensor_tensor(
                    out=rhs[:, :, 0:256],
                    in0=gall[:, :, 0:256],
                    in1=_ap_expand(p16[:], [(2, C), (0, OUT_DIM), (1, 2)]),
                    op=mybir.AluOpType.mult,
                )
                nc.gpsimd.tensor_copy(out=rhs[:, :, 256:258], in_=p16[:])

                # ---- self-edge rhs (f32) ----
                fb32 = fb32p.tile([P, 258], F32, tag="fb32")
                nc.gpsimd.tensor_copy(out=fb32[:, 0:256], in_=fbg[:, oi, 0:256])
                nc.gpsimd.memset(fb32[:, 256:258], 1.0)

                # ---- aggregation matmuls ----
                ps = aggps.tile([P, 258], F32, tag="agg")
                nc.tensor.matmul(
                    out=ps[:], lhsT=epsI_t[:], rhs=fb32[:], start=True, stop=False
                )
                for c in range(C):
                    nc.tensor.matmul(
                        out=ps[:],
                        lhsT=oneh[:, :, c],
                        rhs=rhs[:, c, :],
                        start=False,
                        stop=(c == C - 1),
                    )

                # ---- finalize ----
                rcp = ep.tile([P, 2], F32, tag="rcp")
                nc.vector.reciprocal(out=rcp[:], in_=ps[:, 256:258])
                for h in range(2):
                    in_ap = _ap_expand(ps[:], [(2, OUT_DIM)])
                    in_ap = bass.AP(in_ap.tensor, in_ap.offset + h, in_ap.ap)
                    nc.scalar.activation(
                        out=og[:, oi, h * OUT_DIM : (h + 1) * OUT_DIM],
                        in_=in_ap,
                        func=mybir.ActivationFunctionType.Copy,
                        scale=rcp[:, h : h + 1],
                    )

                if oi == OG - 1 or t == ntiles - 1:
                    t0 = t - oi
                    nc.sync.dma_start(
                        out=out_d[t0 * P : (t + 1) * P, :].rearrange(
                            "(g p) c -> p g c", p=P
                        ),
                        in_=og[:, 0 : oi + 1, :],
                    )

    nc.compile()
    return nc


_prog_cache = {}


def kernel(x, edge_index, W_w, W_b, a):
    cfg, shared, per_core = host_prep(x, edge_index, W_w, W_b, a, n_cores=8)
    if cfg not in _prog_cache:
        _prog_cache[cfg] = build_program(cfg)
    nc = _prog_cache[cfg]
    in_maps = [
        {kk: v for kk, v in {**shared, **pc}.items() if not kk.startswith("_")}
        for pc in per_core
    ]
    res = run_bass_kernel_spmd(nc, in_maps, list(range(cfg.n_cores)))
    out = np.zeros((cfg.n_nodes, 2 * OUT_DIM), dtype=np.float32)
    for k in range(cfg.n_cores):
        pc = per_core[k]
        out[pc["_nodes"]] = res.results[k]["out"][pc["_rows"]].astype(np.float32)
    return out



# revision 4
# speedup vs baseline: 3.3252x; 3.3252x over previous
"""GAT layer kernel for Trainium2, 8 NeuronCores.

Changes vs v4:
  - tc.strict_bb_all_engine_barrier() between table build and phase 2:
    the DRAM whaug round-trip is NOT hazard-tracked by Tile (SBUF/PSUM
    only), so phase-2 gathers raced phase-1 writes in v3/v4.
  - Phase 2 modulo-scheduled (3 stages): iter i emits agg/finalize for
    tile i-2, oneh/scores/rhs for tile i-1, gathers for tile i. Keeps
    every engine stream free of intra-tile long dependency chains.
  - gpsimd stream = gathers only (den copy -> ACT, fb32 ones -> DVE).
  - SWDGE ring enlarged (dynamic_dma_scratch_size) + 2 SWDGE queues,
    gathers round-robin queues so desc-gen overlaps drains.
"""

import math
import sys
from dataclasses import dataclass

import numpy as np

sys.path.insert(0, "/opt/trn_rl_repo")

import concourse.bass as bass
import concourse.mybir as mybir
import concourse.tile as tile
from concourse import bacc
from concourse.masks import make_identity
from concourse.bass_utils import run_bass_kernel_spmd

N_NODES = 50000
IN_DIM = 256
OUT_DIM = 128
P = 128
TW = 256  # table row stride (f16 elems) = 512B
SPLIT = 32768
SHIFT = 4.0
EPS_SELF = 1e-20

F32 = mybir.dt.float32
F16 = mybir.dt.float16
I16 = mybir.dt.int16


@dataclass(frozen=True)
class Cfg:
    n_nodes: int
    n_cores: int
    CA: int
    CB: int
    tcol: int = 0  # column of t0 in the gathered row (= 2*dstar)
    mode: str = "full"  # "full" | "gather_only"
    queues: int = 4
    scratch: int = 65536
    gelem: int = TW  # gather elem_size (f16 elems); < TW reads row prefix
    gmax: int = 8  # max chunks (128 idxs each) per gather instruction
    gsplit: int = 1  # 1: one 512B desc/row; 2: two 256B descs/row
    negpad: int = 0  # 1: -1 idx padding + per-core valid-count registers
    span_tiles: int = 8
    ogrp: int = 8
    reps: int = 1

    @property
    def C(self):
        return self.CA + self.CB

    @property
    def nodes_per_core(self):
        return self.n_nodes // self.n_cores

    @property
    def ntiles(self):
        return (self.nodes_per_core + P - 1) // P

    @property
    def npad(self):
        return self.n_cores * self.ntiles * P


def _ap_expand(ap, dims):
    return bass.AP(ap.tensor, ap.offset, [list(ap.ap[0])] + [[s, c] for s, c in dims])


def _wrap_rep(idxs):
    """flat int16 list (len%128==0) -> [128, n/16] wrapped + replicated."""
    n = len(idxs)
    blk = np.asarray(idxs, dtype=np.int16).reshape(n // 16, 16).T
    return np.tile(blk, (8, 1))


def host_prep(x, edge_index, W_w, W_b, a, n_cores=8):
    x = np.asarray(x, dtype=np.float32)
    edge_index = np.asarray(edge_index)
    W_w = np.asarray(W_w, dtype=np.float32)
    W_b = np.asarray(W_b, dtype=np.float32)
    a = np.asarray(a, dtype=np.float32)
    assert np.abs(W_b).max() == 0.0

    n_nodes, in_dim = x.shape
    D = OUT_DIM
    n_edges = edge_index.shape[1]

    a_src, a_dst = a[:D], a[D:]
    dstar = int(np.argmax(np.abs(a_dst)))
    W_int = (
        W_w.reshape(in_dim, 2, D).transpose(0, 2, 1).reshape(in_dim, 2 * D)
    )
    ws0 = W_w[:, 0:D] @ a_src
    ws1 = W_w[:, D:] @ a_src
    wt0 = W_w[:, 0:D] @ a_dst
    wt1 = W_w[:, D:] @ a_dst
    W_V = W_int.copy()
    W_V[:, 2 * dstar] = wt0
    W_V[:, 2 * dstar + 1] = wt1
    wbig = np.concatenate(
        [W_V, ws0[:, None], ws1[:, None]], axis=1
    ).astype(np.float16)  # [in_dim, 258]

    src = np.asarray(edge_index[0], dtype=np.int64)
    dst = np.asarray(edge_index[1], dtype=np.int64)

    npc = n_nodes // n_cores
    ntiles = (npc + P - 1) // P

    # LPT: node -> (global tile, slot), balancing per-tile edge count
    import heapq

    ntile_tot = n_cores * ntiles
    deg_all = np.bincount(src, minlength=n_nodes)
    order_n = np.argsort(-deg_all, kind="stable")
    heap = [(0, t) for t in range(ntile_tot)]
    heapq.heapify(heap)
    fill = np.zeros(ntile_tot, dtype=np.int64)
    node_tile = np.zeros(n_nodes, dtype=np.int64)
    node_slot = np.zeros(n_nodes, dtype=np.int64)
    for n in order_n:
        while True:
            w, t = heapq.heappop(heap)
            if fill[t] < P:
                break
        node_tile[n] = t
        node_slot[n] = fill[t]
        fill[t] += 1
        if fill[t] < P:
            heapq.heappush(heap, (w + int(deg_all[n]), t))

    grow = node_tile * P + node_slot  # node -> global table row
    npad = ntile_tot * P

    # per-edge: owner core = src's core; global row ids
    ecore = node_tile[src] // ntiles
    etile_l = node_tile[src] % ntiles  # local tile on owner core
    eslot = node_slot[src]
    edst_grow = grow[dst]

    # per-core/per-tile/per-class edge counts -> CA, CB (global maxima)
    erow_l = np.zeros(n_edges, dtype=np.int64)
    for k in range(n_cores):
        m = ecore == k
        erow_l[m] = (edst_grow[m] - k * ntiles * P) % npad
    eclassB = erow_l >= SPLIT

    cntA = np.zeros((n_cores, ntiles), dtype=np.int64)
    cntB = np.zeros((n_cores, ntiles), dtype=np.int64)
    np.add.at(cntA, (ecore[~eclassB], etile_l[~eclassB]), 1)
    np.add.at(cntB, (ecore[eclassB], etile_l[eclassB]), 1)
    CA = int(math.ceil(cntA.max() / P))
    CB = int(math.ceil(cntB.max() / P))
    cfg = Cfg(n_nodes=n_nodes, n_cores=n_cores, CA=CA, CB=CB, tcol=2 * dstar)
    C = cfg.C

    # iota constants
    iota_sc = np.broadcast_to(
        np.arange(P, dtype=np.float16)[None, :, None], (P, P, C)
    ).copy()  # [p, s, c] = s  (for oneh)
    shared = {"wbig": wbig, "iota_sc": iota_sc}
    per_core = []
    for k in range(n_cores):
        # rotated node -> local row
        lrow_node = (grow - k * ntiles * P) % npad  # node -> local row
        # xT rotated: local row r holds node with lrow_node == r
        xT = np.zeros((in_dim, npad), dtype=np.float16)
        own = lrow_node  # [n_nodes]
        xT[:, own] = x.T.astype(np.float16)

        m = ecore == k
        et, es = etile_l[m], eslot[m]
        er = erow_l[m]
        eB = eclassB[m]

        # slot assignment within tile: A edges then B edges
        srcL = np.full((ntiles, C * P), -1.0, dtype=np.float16)
        idxA = np.zeros((ntiles, CA * P), dtype=np.int64)
        idxB = np.zeros((ntiles, CB * P), dtype=np.int64)
        vcntA = np.zeros(ntiles, dtype=np.int64)
        vcntB = np.zeros(ntiles, dtype=np.int64)
        order = np.lexsort((er, eB, et))  # group by tile, class A first
        et, es, er, eB = et[order], es[order], er[order], eB[order]
        for t in range(ntiles):
            tm = et == t
            rA = er[tm & ~eB]
            sA = es[tm & ~eB]
            rB = er[tm & eB] - SPLIT
            sB = es[tm & eB]
            idxA[t, : len(rA)] = rA
            idxB[t, : len(rB)] = rB
            vcntA[t] = len(rA)
            vcntB[t] = len(rB)
            # slot s of region -> (chunk s//P within region, partition s%P)
            a_sl = np.arange(len(rA))
            srcL[t, (a_sl // P) * P + a_sl % P] = sA
            b_sl = np.arange(len(rB))
            srcL[t, CA * P + (b_sl // P) * P + b_sl % P] = sB

        # dma_gather order: idx i -> out (partition i%128, block i//128);
        # slot (chunk c, partition p) = flat c*128+p = i  => identity order
        idxA16 = np.stack([_wrap_rep(idxA[t]) for t in range(ntiles)], axis=1)
        idxB16 = np.stack([_wrap_rep(idxB[t]) for t in range(ntiles)], axis=1)
        # [128, ntiles, n/16] -> [128, ntiles * n/16]
        idxA16 = np.ascontiguousarray(idxA16).reshape(P, -1)
        idxB16 = np.ascontiguousarray(idxB16).reshape(P, -1)

        # per-instruction valid counts (windows of gmax*P idxs); empty
        # windows get one row-0 idx so the ucode never sees 0 valids
        gmax = 8
        cnt_rows = []
        for t in range(ntiles):
            for CX, vc, tab in ((CA, vcntA, idxA), (CB, vcntB, idxB)):
                done = 0
                while done < CX:
                    an = min(gmax, CX - done)
                    v = int(np.clip(vc[t] - done * P, 0, an * P))
                    if v == 0:
                        tab[t, done * P] = 0
                        v = 1
                    cnt_rows.append(v)
                    done += an
        cnts = np.asarray(cnt_rows, dtype=np.int32).reshape(1, -1)

        # srcL per-slot in [P, ntiles*C] layout (partition = slot%P)
        srcL_pc = np.ascontiguousarray(
            srcL.reshape(ntiles, C, P).transpose(2, 0, 1)
        ).reshape(P, ntiles * C)
        mine = np.nonzero(node_tile // ntiles == k)[0]
        rows_k = lrow_node[mine]
        per_core.append(
            {
                "xT": xT,
                "idxA": idxA16.astype(np.int16),
                "idxB": idxB16.astype(np.int16),
                "srcL": srcL_pc,
                "cnts": cnts,
                "_nodes": mine,
                "_rows": rows_k,
                "_adst": a_dst.astype(np.float32),
                "_dstar": dstar,
            }
        )
    return cfg, shared, per_core


def build_program(cfg: Cfg):
    CA, CB, C = cfg.CA, cfg.CB, cfg.C
    ntiles, npad = cfg.ntiles, cfg.npad
    OG = cfg.ogrp
    nc = bacc.Bacc(
        "TRN2",
        target_bir_lowering=False,
        debug=False,
        dynamic_dma_scratch_size=cfg.scratch,
        num_swdge_queues=cfg.queues,
    )

    xT_d = nc.dram_tensor("xT", [IN_DIM, npad], F16, kind="ExternalInput")
    wbig_d = nc.dram_tensor("wbig", [IN_DIM, 258], F16, kind="ExternalInput")
    iosc_d = nc.dram_tensor("iota_sc", [P, P, C], F16, kind="ExternalInput")
    idxA_d = nc.dram_tensor("idxA", [P, ntiles * CA * 8], I16, kind="ExternalInput")
    idxB_d = nc.dram_tensor("idxB", [P, ntiles * CB * 8], I16, kind="ExternalInput")
    srcL_d = nc.dram_tensor("srcL", [P, ntiles * C], F16, kind="ExternalInput")
    n_ginst = ntiles * (
        -(-CA // cfg.gmax) + -(-CB // cfg.gmax)
    )
    cnts_d = nc.dram_tensor("cnts", [1, n_ginst], mybir.dt.int32, kind="ExternalInput")
    out_d = nc.dram_tensor("out", [ntiles * P, 2 * OUT_DIM], F16, kind="ExternalOutput")

    whaug_d = nc.dram_tensor("whaug", [npad, TW], F16)

    n_alltiles = npad // P

    with tile.TileContext(nc) as tc:
        with (
            tc.tile_pool(name="const", bufs=1) as constp,
            tc.tile_pool(name="xk", bufs=2) as xkp,
            tc.tile_pool(name="bld_ps", bufs=2, space="PSUM") as bldps,
            tc.tile_pool(name="augg", bufs=2) as auggp,
            tc.tile_pool(name="tr_ps", bufs=2, space="PSUM") as trps,
            tc.tile_pool(name="oneT", bufs=2) as oneTp,
            tc.tile_pool(name="fb32", bufs=2) as fb32p,
            tc.tile_pool(name="gall", bufs=3) as gallp,
            tc.tile_pool(name="oneh", bufs=2) as onehp,
            tc.tile_pool(name="rhs", bufs=2) as rhsp,
            tc.tile_pool(name="p16", bufs=2) as p16p,
            tc.tile_pool(name="ework", bufs=2) as ep,
            tc.tile_pool(name="agg_ps", bufs=2, space="PSUM") as aggps,
            tc.tile_pool(name="s_ps", bufs=2, space="PSUM") as spsp,
            tc.tile_pool(name="og", bufs=2) as ogp,
        ):
            # ---------------- constants ----------------
            wb = constp.tile([P, 2, 258], F16, tag="wb")
            nc.sync.dma_start(
                out=wb[:], in_=wbig_d[:, :].rearrange("(kt kp) c -> kp kt c", kp=P)
            )
            iosc_t = constp.tile([P, P, C], F16, tag="iosc")
            nc.sync.dma_start(out=iosc_t[:], in_=iosc_d[:, :, :])
            idxA_t = constp.tile([P, ntiles * CA * 8], I16, tag="idxA")
            nc.sync.dma_start(out=idxA_t[:], in_=idxA_d[:, :])
            idxB_t = constp.tile([P, ntiles * CB * 8], I16, tag="idxB")
            nc.sync.dma_start(out=idxB_t[:], in_=idxB_d[:, :])
            srcL_t = constp.tile([P, ntiles * C], F16, tag="srcL")
            nc.sync.dma_start(out=srcL_t[:], in_=srcL_d[:, :])
            cnts_t = constp.tile([1, n_ginst], mybir.dt.int32, tag="cnts")
            nc.sync.dma_start(out=cnts_t[:], in_=cnts_d[:, :])
            gregs = [
                nc.gpsimd.alloc_register(name=f"gcnt{i}") for i in range(4)
            ]
            shift_t = constp.tile([P, 1], F32, tag="shift")
            nc.vector.memset(shift_t[:], -SHIFT)
            ident_t = constp.tile([P, P], F32, tag="ident")
            make_identity(nc, ident_t[:])
            ident16_t = constp.tile([P, P], F16, tag="ident16")
            nc.vector.tensor_copy(out=ident16_t[:], in_=ident_t[:])
            epsI_t = constp.tile([P, P], F32, tag="epsI")
            nc.vector.tensor_scalar(
                out=epsI_t[:], in0=ident_t[:], scalar1=EPS_SELF, scalar2=None,
                op0=mybir.AluOpType.mult,
            )
            own_t = constp.tile([P, ntiles, 258], F16, tag="own")

            # ---------------- phase 1: build whaug table ----------------
            GRP = 8
            n0 = 0
            ci = 0
            while n0 < n_alltiles:
                span = min(cfg.span_tiles, n_alltiles - n0)
                xk = xkp.tile([P, 2, cfg.span_tiles * P], F16, tag="xk")
                for kt in range(2):
                    nc.sync.dma_start(
                        out=xk[:, kt, 0 : span * P],
                        in_=xT_d[kt * P : (kt + 1) * P, n0 * P : (n0 + span) * P],
                    )
                g0 = 0
                while g0 < span:
                    grp = min(GRP, span - g0)
                    nt0 = n0 + g0
                    is_own = nt0 < ntiles
                    if is_own:
                        grp = min(grp, ntiles - nt0)  # own group never crosses
                    else:
                        aug = auggp.tile([P, GRP, TW], F16, tag="aug")
                    for g in range(grp):
                        nt = nt0 + g
                        ps = bldps.tile([P, 258], F32, tag="bld")
                        for kt in range(2):
                            nc.tensor.matmul(
                                out=ps[:],
                                lhsT=xk[:, kt, (nt - n0) * P : (nt - n0 + 1) * P],
                                rhs=wb[:, kt, :],
                                start=(kt == 0),
                                stop=(kt == 1),
                            )
                        dst_ap = (
                            own_t[:, nt, :] if is_own else aug[:, g, 0:TW]
                        )
                        src_ap = ps[:] if is_own else ps[:, 0:TW]
                        if ci % 2 == 0:
                            nc.vector.tensor_copy(out=dst_ap, in_=src_ap)
                        else:
                            nc.scalar.activation(
                                out=dst_ap, in_=src_ap,
                                func=mybir.ActivationFunctionType.Copy,
                            )
                        ci += 1
                    r0 = nt0 * P
                    if is_own:
                        nc.sync.dma_start(
                            out=whaug_d[r0 : r0 + grp * P, :].rearrange(
                                "(g p) c -> p g c", p=P
                            ),
                            in_=own_t[:, nt0 : nt0 + grp, 0:TW],
                        )
                    else:
                        nc.sync.dma_start(
                            out=whaug_d[r0 : r0 + grp * P, :].rearrange(
                                "(g p) c -> p g c", p=P
                            ),
                            in_=aug[:, 0:grp, :],
                        )
                    g0 += grp
                n0 += span

            # whaug (DRAM) is not hazard-tracked by Tile: force phase-1
            # writes to complete before any phase-2 gather reads it.
            tc.strict_bb_all_engine_barrier()

            # ---------------- phase 2 (modulo-scheduled) ----------------
            state = {}  # tile -> dict of live tiles
            qi = [0]

            GS = cfg.gsplit
            GE = cfg.gelem // GS
            ipt = -(-CA // cfg.gmax) + -(-CB // cfg.gmax)  # instrs per tile
            if cfg.negpad:
                # host builds count windows with gmax=8 and guarantees the
                # first A-window is fully valid (row-0 filled)
                assert cfg.gmax == 8 and CA >= cfg.gmax and cfg.gsplit == 1

            def emit_gather(t):
                if cfg.negpad and ipt > 1:
                    k0 = t * ipt
                    nc.gpsimd.reg_load(
                        gregs[: ipt - 1], cnts_t[0:1, k0 + 1 : k0 + ipt]
                    )
                gwin = [0]

                def _gcount():
                    w = gwin[0]
                    gwin[0] += 1
                    if w == 0:
                        return cfg.gmax * P if CA >= cfg.gmax else CA * P
                    return gregs[w - 1]
                gall = gallp.tile([P, GS, C, GE], F16, tag="gall")
                for h in range(GS):
                    a_done = 0
                    while a_done < CA:
                        an = min(cfg.gmax, CA - a_done)
                        nc.gpsimd.dma_gather(
                            out_ap=gall[:, h, a_done : a_done + an, :],
                            in_ap=whaug_d[0:SPLIT, h * GE : (h + 1) * GE],
                            idxs_ap=idxA_t[
                                :,
                                (t * CA + a_done) * 8 : (t * CA + a_done + an) * 8,
                            ],
                            num_idxs=an * P,
                            num_idxs_reg=_gcount() if cfg.negpad else an * P,
                            elem_size=GE,
                            elem_step=TW,
                            queue_num=qi[0] % cfg.queues,
                        )
                        qi[0] += 1
                        a_done += an
                    b_done = 0
                    while b_done < CB:
                        bn = min(cfg.gmax, CB - b_done)
                        nc.gpsimd.dma_gather(
                            out_ap=gall[:, h, CA + b_done : CA + b_done + bn, :],
                            in_ap=whaug_d[SPLIT:npad, h * GE : (h + 1) * GE],
                            idxs_ap=idxB_t[
                                :,
                                (t * CB + b_done) * 8 : (t * CB + b_done + bn) * 8,
                            ],
                            num_idxs=bn * P,
                            num_idxs_reg=_gcount() if cfg.negpad else bn * P,
                            elem_size=GE,
                            elem_step=TW,
                            queue_num=qi[0] % cfg.queues,
                        )
                        qi[0] += 1
                        b_done += bn
                state[t] = {"gall": gall}

            def emit_front(t):
                st = state[t]
                gall = st["gall"]
                oneh = onehp.tile([P, P, C], F16, tag="oneh")
                nc.vector.tensor_tensor(
                    out=oneh[:],
                    in0=_ap_expand(srcL_t[:, t * C : (t + 1) * C], [(0, P), (1, C)]),
                    in1=iosc_t[:],
                    op=mybir.AluOpType.is_equal,
                )
                fb32 = fb32p.tile([P, 258], F32, tag="fb32")
                nc.scalar.activation(
                    out=fb32[:, 0:256], in_=own_t[:, t, 0:256],
                    func=mybir.ActivationFunctionType.Copy,
                )
                nc.vector.memset(fb32[:, 256:258], 1.0)

                s_ps = spsp.tile([P, C, 2], F32, tag="s_ps")
                g0 = 0
                gi = 0
                while g0 < C:
                    gn = min(8, C - g0)
                    trp = trps.tile([P, 8, P], F16, tag="oneT_ps")
                    for c in range(g0, g0 + gn):
                        nc.tensor.transpose(
                            out=trp[:, c - g0, :], in_=oneh[:, :, c],
                            identity=ident16_t[:],
                        )
                    oneT = oneTp.tile([P, 8, P], F16, tag="oneT")
                    if gi % 2 == 0:
                        nc.vector.tensor_copy(
                            out=oneT[:, 0:gn, :], in_=trp[:, 0:gn, :]
                        )
                    else:
                        nc.scalar.activation(
                            out=oneT[:, 0:gn, :], in_=trp[:, 0:gn, :],
                            func=mybir.ActivationFunctionType.Copy,
                        )
                    for c in range(g0, g0 + gn):
                        nc.tensor.matmul(
                            out=s_ps[:, c, :],
                            lhsT=oneT[:, c - g0, :],
                            rhs=own_t[:, t, 256:258],
                            start=True,
                            stop=True,
                        )
                    g0 += gn
                    gi += 1

                e_t = ep.tile([P, C, 2], F32, tag="e_t")
                thv, tof = cfg.tcol // GE, cfg.tcol % GE
                nc.vector.tensor_tensor(
                    out=e_t[:],
                    in0=s_ps[:],
                    in1=gall[:, thv, :, tof : tof + 2],
                    op=mybir.AluOpType.add,
                )
                e_s = ep.tile([P, C, 2], F32, tag="e_s")
                nc.vector.tensor_scalar(
                    out=e_s[:], in0=e_t[:], scalar1=0.2, scalar2=None,
                    op0=mybir.AluOpType.mult,
                )
                lr_t = ep.tile([P, C, 2], F32, tag="lr_t")
                nc.vector.tensor_tensor(
                    out=lr_t[:], in0=e_t[:], in1=e_s[:], op=mybir.AluOpType.max,
                )
                if cfg.negpad:
                    nc.vector.tensor_scalar(
                        out=lr_t[:], in0=lr_t[:], scalar1=12.0, scalar2=None,
                        op0=mybir.AluOpType.min,
                    )
                p16 = p16p.tile([P, C, 2], F16, tag="p16")
                nc.scalar.activation(
                    out=p16[:], in_=lr_t[:],
                    func=mybir.ActivationFunctionType.Exp,
                    bias=shift_t[:, 0:1],
                )

                rhs = rhsp.tile([P, C, 258], F16, tag="rhs")
                for h in range(GS):
                    nc.vector.tensor_tensor(
                        out=rhs[:, :, h * GE : (h + 1) * GE],
                        in0=gall[:, h, :, :],
                        in1=_ap_expand(p16[:], [(2, C), (0, GE // 2), (1, 2)]),
                        op=mybir.AluOpType.mult,
                    )
                nc.scalar.activation(
                    out=rhs[:, :, 256:258], in_=p16[:],
                    func=mybir.ActivationFunctionType.Copy,
                )
                st.update(oneh=oneh, fb32=fb32, rhs=rhs)

            def emit_back(t, og_state):
                st = state.pop(t)
                oi = t % OG
                if oi == 0:
                    og_new = ogp.tile([P, OG, 2 * OUT_DIM], F16, tag="og")
                    og_state[0] = og_new
                og = og_state[0]

                ps = aggps.tile([P, 258], F32, tag="agg")
                nc.tensor.matmul(
                    out=ps[:], lhsT=epsI_t[:], rhs=st["fb32"][:],
                    start=True, stop=False,
                )
                for c in range(C):
                    nc.tensor.matmul(
                        out=ps[:],
                        lhsT=st["oneh"][:, :, c],
                        rhs=st["rhs"][:, c, :],
                        start=False,
                        stop=(c == C - 1),
                    )

                rcp = ep.tile([P, 2], F32, tag="rcp")
                nc.vector.reciprocal(out=rcp[:], in_=ps[:, 256:258])
                for h in range(2):
                    in_ap = _ap_expand(ps[:], [(2, OUT_DIM)])
                    in_ap = bass.AP(in_ap.tensor, in_ap.offset + h, in_ap.ap)
                    nc.scalar.activation(
                        out=og[:, oi, h * OUT_DIM : (h + 1) * OUT_DIM],
                        in_=in_ap,
                        func=mybir.ActivationFunctionType.Copy,
                        scale=rcp[:, h : h + 1],
                    )
                if oi == OG - 1 or t == ntiles - 1:
                    t0 = t - oi
                    nc.sync.dma_start(
                        out=out_d[t0 * P : (t + 1) * P, :].rearrange(
                            "(g p) c -> p g c", p=P
                        ),
                        in_=og[:, 0 : oi + 1, :],
                    )

            if cfg.negpad:
                for _ in range(3):
                    gz = gallp.tile([P, GS, C, GE], F16, tag="gall")
                    nc.vector.memset(gz[:], 0.0)

            for _ in range(cfg.reps):
                og_state = [None]
                if cfg.mode == "gather_only":
                    for t in range(ntiles):
                        emit_gather(t)
                    state.clear()
                else:
                    for i in range(ntiles + 2):
                        if i >= 2:
                            emit_back(i - 2, og_state)
                        if 1 <= i <= ntiles:
                            emit_front(i - 1)
                        if i < ntiles:
                            emit_gather(i)

    nc.compile()
    return nc


_prog_cache = {}


def core_out(res_k, pc):
    o = res_k["out"][pc["_rows"]].astype(np.float32)
    w = np.asarray(pc["_adst"], dtype=np.float32)
    ds = int(pc["_dstar"])
    wd = float(w[ds])
    for h in range(2):
        B = o[:, h * OUT_DIM : (h + 1) * OUT_DIM]
        tagg = B[:, ds].copy()
        B[:, ds] = (tagg * (1.0 + wd) - B @ w) / wd
    return o


def kernel(x, edge_index, W_w, W_b, a):
    cfg, shared, per_core = host_prep(x, edge_index, W_w, W_b, a, n_cores=8)
    if cfg not in _prog_cache:
        _prog_cache[cfg] = build_program(cfg)
    nc = _prog_cache[cfg]
    in_maps = [
        {kk: v for kk, v in {**shared, **pc}.items() if not kk.startswith("_")}
        for pc in per_core
    ]
    res = run_bass_kernel_spmd(nc, in_maps, list(range(cfg.n_cores)))
    out = np.zeros((cfg.n_nodes, 2 * OUT_DIM), dtype=np.float32)
    for k in range(cfg.n_cores):
        pc = per_core[k]
        out[pc["_nodes"]] = core_out(res.results[k], pc)
    return out
